# revision 1
# baseline (speedup 1.0000x reference)
"""Causal multi-head attention (B=2, T=2048, D=1024, H=16) on 8 TRN2 NeuronCores.

Sharding: core c = (batch b = c//4, head-group g = c%4). Each core owns 4 heads
(= 256 contiguous dims of D) of one batch: Megatron-style tensor parallelism on
heads x data parallelism on batch. Per-core partial output projections are
summed with chunked on-chip ReduceScatters over each batch's 4 cores; the host
only re-assembles the resulting shards.

Device-side layout choices (host pre-transposes, pure data movement):
  - xT  [D, T]        = x[b].T so projections contract D on the partition dim.
  - qT/kT [256, T]    computed directly transposed (dims on partitions).
  - scoresT[k, q]     = k @ qT -> softmax runs in the k-on-partitions domain,
                        so the AV matmul (lhsT=v, rhs=attnT) needs no T x T
                        transpose anywhere.
  - v_aug [k, 4*65]   v with a ones column appended per head: AV then yields
                        yT' [65, span] whose row 64 is the softmax denominator.
  - softmax: exp(s) without row-max subtraction (scores are O(1): the q,k
    projections are variance-1, scale 1/8 folded into Wq host-side), causal
    tile classification (full-skip / full-keep / diagonal-with-mask-values).
  - normalization: per-span stacked reciprocal on DVE, broadcast across
    partitions via a PE rank-1 outer product, applied during the PSUM->SBUF
    evacuation of yT'.
  - per-q-span pipeline: attention -> normalize -> out-projection -> chunked
    ReduceScatter -> output DMA, so collectives overlap the next span.

Dtypes: all matmul operands run in bf16 (1.0 PE cycles/row; f32r would be 1.5)
with fp32 PSUM accumulation throughout; biases are added in fp32 during PSUM
evacuation. The softmax normalization cancels most of the correlated bf16
quantization error: measured end-to-end relative error is ~5.9e-3 vs the fp32
reference (absmax ~0.4% of the output scale), verified identically in CoreSim
and on hardware.
"""

import os
import numpy as np
import ml_dtypes

BF16 = ml_dtypes.bfloat16

B, T, D, H = 2, 2048, 1024, 16
HD = D // H                     # 64
NCORES = 8
GROUPS = 4                      # cores per batch (tensor-parallel degree)
HL = H // GROUPS                # heads per core = 4
DL = D // GROUPS                # dims per core = 256
SP = 512                        # free-dim span per matmul (one PSUM bank, fp32)
QS = T // SP                    # 4 q spans
KT = T // 128                   # 16 k tiles
RS_ROWS = T // GROUPS           # 512 rows per ReduceScatter chunk
SCALE = HD ** -0.5

_CACHE = {}


def _build_program():
    import concourse.bass as bass  # noqa: F401  (registers bass machinery)
    import concourse.tile as tile
    from concourse import bacc, mybir

    f32 = mybir.dt.float32
    f32r = mybir.dt.float32r
    bf16 = mybir.dt.bfloat16
    Exp = mybir.ActivationFunctionType.Exp
    Identity = mybir.ActivationFunctionType.Identity

    nc = bacc.Bacc("TRN2", target_bir_lowering=False, debug=False,
                   num_devices=NCORES)

    xT = nc.dram_tensor("xT", [D, T], bf16, kind="ExternalInput")
    wqT = nc.dram_tensor("wqT", [D, DL], bf16, kind="ExternalInput")
    wkT = nc.dram_tensor("wkT", [D, DL], bf16, kind="ExternalInput")
    wvT = nc.dram_tensor("wvT", [D, DL], bf16, kind="ExternalInput")
    woT = nc.dram_tensor("woT", [DL, D], bf16, kind="ExternalInput")
    bqP = nc.dram_tensor("bqP", [128, 2], f32, kind="ExternalInput")
    bkP = nc.dram_tensor("bkP", [128, 2], f32, kind="ExternalInput")
    bv = nc.dram_tensor("bv", [1, DL], bf16, kind="ExternalInput")
    bo = nc.dram_tensor("bo", [1, D], bf16, kind="ExternalInput")
    maskd = nc.dram_tensor("maskd", [KT, 128, SP], bf16, kind="ExternalInput")
    onesd = nc.dram_tensor("onesd", [128, SP], f32r, kind="ExternalInput")
    onesb = nc.dram_tensor("onesb", [128, SP], bf16, kind="ExternalInput")
    out_ext = nc.dram_tensor("out", [QS, 128, D], f32, kind="ExternalOutput")

    with tile.TileContext(nc) as tc:
        with tc.tile_pool(name="main", bufs=1) as main, \
             tc.tile_pool(name="dram", bufs=1, space="DRAM") as dram:
            qT_s = main.tile([128, 2, T], bf16)
            kT_s = main.tile([128, 2, T], bf16)
            v_s = main.tile([128, KT, HL * 65], bf16)
            yT_s = main.tile([128, 2, T], bf16)
            woT_s = main.tile([128, 2, D], bf16)
            bq_s = main.tile([128, 2], f32)
            bk_s = main.tile([128, 2], f32)
            bv_s = main.tile([1, DL], bf16)
            bo_s = main.tile([1, D], bf16)
            ones_s = main.tile([128, SP], f32r)
            onesb_s = main.tile([128, SP], bf16)
            bo_bc = main.tile([128, D], bf16)
            bv_bc = main.tile([128, DL], bf16)
            maskd_s = main.tile([128, KT, SP], bf16)

            # one partial/rs tile pair per q-span: avoids false DRAM-tile
            # dependencies between a span's ReduceScatter and the next
            # span's out-projection DMAs
            partials = [dram.tile([RS_ROWS, D], f32, name=f"partial{i}")
                        for i in range(QS)]
            rs_outs = [dram.tile([128, D], f32, name=f"rsout{i}")
                       for i in range(QS)]

            # tiny high-priority loads on the sync queue
            nc.sync.dma_start(out=bq_s, in_=bqP[:])
            nc.sync.dma_start(out=bk_s, in_=bkP[:])
            # small loads on the scalar queue
            nc.scalar.dma_start(out=ones_s, in_=onesd[:])
            nc.scalar.dma_start(out=onesb_s, in_=onesb[:])
            nc.scalar.dma_start(out=bv_bc, in_=bv[:].to_broadcast([128, DL]))
            nc.scalar.dma_start(out=bo_bc, in_=bo[:].to_broadcast([128, D]))
            # ones column at index 64 of each head's 65-wide block of v_aug:
            # memset the whole tile (bf16 memset is codegen-legal; the v
            # evacuations overwrite the data columns)
            nc.vector.memset(v_s, 1.0)

            # ---------------- phase 1: projections ----------------
            with tc.tile_pool(name="proj", bufs=1) as proj, \
                 tc.tile_pool(name="pj_psum", bufs=3, space="PSUM") as pj_psum:
                xt_s = proj.tile([128, 8, T], bf16)
                wq_s = proj.tile([128, 8, DL], bf16)
                wk_s = proj.tile([128, 8, DL], bf16)
                wv_s = proj.tile([128, 8, DL], bf16)

                # critical path first: wq then the x chunks (split across the
                # sync and gpsimd queues); wk/wv follow behind x on gpsimd
                wq_r = wqT[:].rearrange("(c p) n -> c p n", p=128)
                for c in range(8):
                    nc.sync.dma_start(out=wq_s[:, c, :], in_=wq_r[c])
                xT_r = xT[:].rearrange("(c p) t -> c p t", p=128)
                for c in range(8):
                    eng = nc.sync if c % 2 == 0 else nc.gpsimd
                    eng.dma_start(out=xt_s[:, c, :], in_=xT_r[c])
                # wk/wv on the scalar queue (needed only after qT finishes),
                # followed by the attention/outproj bulk loads
                for w_s, w_d in ((wk_s, wkT), (wv_s, wvT)):
                    w_r = w_d[:].rearrange("(c p) n -> c p n", p=128)
                    for c in range(8):
                        nc.scalar.dma_start(out=w_s[:, c, :], in_=w_r[c])
                for i in range(KT):
                    nc.scalar.dma_start(out=maskd_s[:, i, :], in_=maskd[i])
                woT_r = woT[:].rearrange("(c p) n -> c p n", p=128)
                for c in range(2):
                    nc.scalar.dma_start(out=woT_s[:, c, :], in_=woT_r[c])

                # qT / kT: out[dims-chunk, t-span]; bias added during the
                # PSUM->SBUF evacuation (per-partition scalar)
                for w_s, b_s, dst, use_act in ((wq_s, bq_s, qT_s, True),
                                               (wk_s, bk_s, kT_s, False)):
                    for mc in range(2):
                        for s in range(QS):
                            ps = pj_psum.tile([128, SP], f32, tag="pj")
                            for kc in range(8):
                                nc.tensor.matmul(
                                    ps,
                                    lhsT=w_s[:, kc, mc * 128:(mc + 1) * 128],
                                    rhs=xt_s[:, kc, s * SP:(s + 1) * SP],
                                    start=(kc == 0), stop=(kc == 7))
                            dstv = dst[:, mc, s * SP:(s + 1) * SP]
                            if use_act:
                                nc.scalar.activation(
                                    dstv, ps, Identity,
                                    bias=b_s[:, mc:mc + 1])
                            else:
                                nc.vector.tensor_scalar_add(
                                    dstv, ps, b_s[:, mc:mc + 1])

                # v: natural layout; bias via rank-1 matmul (free-dim bias)
                for mt in range(KT):
                    ps = pj_psum.tile([128, DL], f32, tag="pjv")
                    for kc in range(8):
                        nc.tensor.matmul(
                            ps,
                            lhsT=xt_s[:, kc, mt * 128:(mt + 1) * 128],
                            rhs=wv_s[:, kc, :],
                            start=(kc == 0), stop=(kc == 7))
                    nc.vector.tensor_add(
                        v_s[:, mt, :].rearrange(
                            "p (h d) -> p h d", d=65)[:, :, 0:64],
                        ps.rearrange("p (h d) -> p h d", d=64),
                        bv_bc.rearrange("p (h d) -> p h d", d=64))

            # ---- phase 2: per-span attention, software-pipelined with the
            # previous span's normalize-broadcast + out-projection + RS so
            # the in-order PE queue never waits on the DVE normalize chain
            with tc.tile_pool(name="attn_t", bufs=6) as attn_t, \
                 tc.tile_pool(name="nrm", bufs=2) as nrm, \
                 tc.tile_pool(name="op_sb", bufs=4) as op_sb, \
                 tc.tile_pool(name="sc_psum", bufs=2, space="PSUM") as sc_psum, \
                 tc.tile_pool(name="av_psum", bufs=3, space="PSUM") as av_psum, \
                 tc.tile_pool(name="pp_psum", bufs=3, space="PSUM") as pp_psum:

                def attention_span(qs):
                    # denominator rows at partitions 0/32/64/96 (engine APs
                    # must start 32-aligned); memset keeps unused rows finite
                    den_stack = nrm.tile([97, SP], f32, tag="den")
                    nc.vector.memset(den_stack, 1.0)
                    nkt = 4 * qs + 4  # causal: later k tiles are all-masked
                    for h in range(HL):
                        mc, r0 = divmod(h, 2)
                        r0 *= 64
                        qv = qT_s[r0:r0 + 64, mc, qs * SP:(qs + 1) * SP]
                        yT_ps = av_psum.tile([65, SP], f32, tag="av")
                        for kt in range(nkt):
                            sc = sc_psum.tile([128, SP], f32, tag="sc")
                            nc.tensor.matmul(
                                sc,
                                lhsT=kT_s[r0:r0 + 64, mc,
                                          kt * 128:(kt + 1) * 128],
                                rhs=qv, start=True, stop=True)
                            at = attn_t.tile([128, SP], bf16, tag="at")
                            nc.scalar.activation(at, sc, Exp)
                            if kt >= 4 * qs:  # diagonal tile: apply mask
                                nc.vector.tensor_mul(at, at, maskd_s[:, kt, :])
                            nc.tensor.matmul(
                                yT_ps, lhsT=v_s[:, kt, h * 65:(h + 1) * 65],
                                rhs=at, start=(kt == 0), stop=(kt == nkt - 1))
                        # evacuate yT' (unnormalized) right away so the PSUM
                        # accumulator frees for the next head
                        nc.scalar.copy(
                            yT_s[r0:r0 + 64, mc, qs * SP:(qs + 1) * SP],
                            yT_ps[0:64, :])
                        nc.vector.tensor_copy(den_stack[32 * h:32 * h + 1, :],
                                              yT_ps[64:65, :])
                    # pure-DVE tail: reciprocal + per-head f32r rows for the
                    # PE broadcast (consumed one span later)
                    rec_f = nrm.tile([97, SP], f32, tag="recf")
                    nc.vector.reciprocal(rec_f, den_stack)
                    rec_hs = []
                    for h in range(HL):
                        rec_h = nrm.tile([1, SP], bf16, tag="rech", bufs=8)
                        nc.vector.tensor_copy(rec_h,
                                              rec_f[32 * h:32 * h + 1, :])
                        rec_hs.append(rec_h)
                    return rec_hs

                def pe_post(qs, rec_hs):
                    # broadcast 1/denom across partitions on the PE, then
                    # normalize yT in place
                    for h in range(HL):
                        mc, r0 = divmod(h, 2)
                        r0 *= 64
                        rb = pp_psum.tile([64, SP], f32, tag="pp")
                        nc.tensor.matmul(rb, lhsT=onesb_s[0:1, 0:64],
                                         rhs=rec_hs[h], start=True, stop=True)
                        yv = yT_s[r0:r0 + 64, mc, qs * SP:(qs + 1) * SP]
                        nc.vector.tensor_mul(yv, yv, rb)
                    # out-projection for this span's 4 q-tiles; each 256-row
                    # half's ReduceScatter fires as soon as its 2 q-tiles
                    # are written so the tail chunk starts earlier
                    hr = RS_ROWS // 2
                    for hf in range(2):
                        for qt in range(4 * qs + 2 * hf, 4 * qs + 2 * hf + 2):
                            for ns in range(2):
                                po = pp_psum.tile([128, SP], f32, tag="pp")
                                for kc in range(2):
                                    nc.tensor.matmul(
                                        po,
                                        lhsT=yT_s[:, kc,
                                                  qt * 128:(qt + 1) * 128],
                                        rhs=woT_s[:, kc,
                                                  ns * SP:(ns + 1) * SP],
                                        start=(kc == 0), stop=(kc == 1))
                                ob = op_sb.tile([128, SP], f32, tag="ob")
                                nc.vector.tensor_add(
                                    ob, po, bo_bc[:, ns * SP:(ns + 1) * SP])
                                nc.sync.dma_start(
                                    out=partials[qs][
                                        (qt - 4 * qs) * 128:
                                        (qt - 4 * qs + 1) * 128,
                                        ns * SP:(ns + 1) * SP],
                                    in_=ob)
                        nc.gpsimd.collective_compute(
                            "ReduceScatter", mybir.AluOpType.add,
                            replica_groups=[[0, 1, 2, 3], [4, 5, 6, 7]],
                            ins=[partials[qs][hf * hr:(hf + 1) * hr, :].opt()],
                            outs=[rs_outs[qs][hf * 64:(hf + 1) * 64, :].opt()])
                        nc.sync.dma_start(
                            out=out_ext[qs, hf * 64:(hf + 1) * 64, :],
                            in_=rs_outs[qs][hf * 64:(hf + 1) * 64, :])

                prev = None
                for qs in range(QS):
                    rec_hs = attention_span(qs)
                    if prev is not None:
                        pe_post(prev[0], prev[1])
                    prev = (qs, rec_hs)
                pe_post(prev[0], prev[1])

    nc.compile()
    return nc


def _get_program():
    if "nc" not in _CACHE:
        _CACHE["nc"] = _build_program()
    return _CACHE["nc"]


def _make_in_maps(x, mask, Wq, bq, Wk, bk, Wv, bv, Wo, bo):
    x = np.asarray(x, np.float32)
    mask = np.asarray(mask, bool)
    Wq = np.asarray(Wq, np.float32)
    Wk = np.asarray(Wk, np.float32)
    Wv = np.asarray(Wv, np.float32)
    Wo = np.asarray(Wo, np.float32)
    bq = np.asarray(bq, np.float32)
    bk = np.asarray(bk, np.float32)
    bv = np.asarray(bv, np.float32)
    bo = np.asarray(bo, np.float32)

    zeros_bo = np.zeros((1, D), np.float32)
    in_maps = []
    per_batch = {}
    for b in range(B):
        xTb = np.ascontiguousarray(x[b].T)
        # diagonal mask tiles of mask[b,0].T: index qs*4+j holds
        # maskT[128*(4qs+j) : +128, 512*qs : +512]
        mT = mask[b, 0].T
        md = np.empty((KT, 128, SP), np.float32)
        for qs in range(QS):
            for j in range(4):
                kt = 4 * qs + j
                md[kt] = mT[kt * 128:(kt + 1) * 128,
                            qs * SP:(qs + 1) * SP].astype(np.float32)
        per_batch[b] = (xTb, md)
    for c in range(NCORES):
        b, g = divmod(c, GROUPS)
        sl = slice(g * DL, (g + 1) * DL)
        xTb, md = per_batch[b]
        in_maps.append({
            "xT": xTb.astype(BF16),
            "wqT": np.ascontiguousarray((Wq[sl] * SCALE).T).astype(BF16),
            "wkT": np.ascontiguousarray(Wk[sl].T).astype(BF16),
            "wvT": np.ascontiguousarray(Wv[sl].T).astype(BF16),
            "woT": np.ascontiguousarray(Wo[:, sl].T).astype(BF16),
            "bqP": np.ascontiguousarray((bq[sl] * SCALE).reshape(2, 128).T),
            "bkP": np.ascontiguousarray(bk[sl].reshape(2, 128).T),
            "bv": bv[sl].reshape(1, DL).astype(BF16),
            "bo": (bo.reshape(1, D) if g == 0 else zeros_bo).astype(BF16),
            "maskd": md.astype(BF16),
            "onesd": np.ones((128, SP), np.float32),
            "onesb": np.ones((128, SP), BF16),
        })
    return in_maps


def _capture_profile(nc, in_maps, tmpdir):
    """Run with NTFF capture and process the profile ourselves (the stock
    trace path can't handle the duplicate-executable NTFFs the axon relay
    produces). Returns (results, exec_time_ns|None)."""
    import glob
    import json
    import re
    import subprocess
    from trn_agent_boot.trn_boot import _ntff_profile_via_ctypes
    from concourse import bass2jax

    hook = _ntff_profile_via_ctypes("/opt/axon/libaxon_pjrt.so")
    if hook is None:
        raise RuntimeError("libaxon_pjrt.so lacks NTFF profile symbols")
    os.makedirs(tmpdir, exist_ok=True)
    with hook(tmpdir, [0]):
        results = bass2jax.run_bass_via_pjrt(nc, in_maps, n_cores=NCORES)

    # group NTFF/NEFF pairs by executable id; use the newest executable
    ntffs = glob.glob(os.path.join(tmpdir, "*_body*-device*.ntff"))
    best, best_id = None, -1
    for f in ntffs:
        m = re.search(r"executable(\d+)-device000000", f)
        if m and int(m.group(1)) > best_id:
            best_id, best = int(m.group(1)), f
    if best is None:
        raise RuntimeError(f"no NTFF produced in {tmpdir}")
    neff = re.sub(r"-device\d+-execution-\d+\.ntff$", ".neff", best)
    out_json = os.path.join(tmpdir, "prof.json")
    subprocess.check_call(
        ["neuron-profile", "view", "--ignore-nc-buf-usage", "-s", best,
         "-n", neff, "--output-format=json", f"--output-file={out_json}"],
        cwd=tmpdir)
    summary = json.load(open(out_json))["summary"][0]
    return results, int(summary["total_time"] * 1e9)


def kernel(x, mask, Wq, bq, Wk, bk, Wv, bv, Wo, bo):
    from concourse import bass_utils

    in_maps = _make_in_maps(x, mask, Wq, bq, Wk, bk, Wv, bv, Wo, bo)
    nc = _get_program()

    trace = bool(int(os.environ.get("MHA_TRACE", "0")))
    tmpdir = os.environ.get("MHA_TRACE_DIR") or None
    results = None
    if trace and tmpdir:
        try:
            results, exec_ns = _capture_profile(nc, in_maps, tmpdir)
            _CACHE["last_exec_time_ns"] = exec_ns
        except Exception as e:  # profiling is best-effort
            print(f"profiling unavailable: {type(e).__name__}: {e}")
            results = None
    if results is None:
        results = bass_utils.run_bass_kernel_spmd(
            nc, in_maps, core_ids=list(range(NCORES))).results
        _CACHE.setdefault("last_exec_time_ns", None)

    out = np.empty((B, T, D), np.float32)
    for c in range(NCORES):
        b, rk = divmod(c, GROUPS)
        o = results[c]["out"]
        for qs in range(QS):  # each span was reduce-scattered in two halves
            for hf in range(2):
                lo = qs * RS_ROWS + hf * (RS_ROWS // 2) + rk * 64
                out[b, lo:lo + 64] = o[qs, hf * 64:(hf + 1) * 64]
    return out



# revision 5
# speedup vs baseline: 1.2599x; 1.2599x over previous
"""Causal multi-head attention (B=2, T=2048, D=1024, H=16) on 8 TRN2 NeuronCores.

Sharding: core c = (batch b = c//4, head-group g = c%4). Each core owns 4 heads
(= 256 contiguous dims of D) of one batch: Megatron-style tensor parallelism on
heads x data parallelism on batch.

v2 changes over the ReduceScatter baseline (352us):
  - Out-projection reduction replaced by a per-q-span 8-way bf16 AllToAll of
    the normalized attention output yT. Rank r's "territory" is the 64-col
    q-block r of each span, for BOTH batches; every core has real data for
    every peer (its dims-slice of its batch), so the exchange is SPMD-uniform
    with no junk shards. After the A2A each core holds full-D yT for its 64 q
    columns x 2 batches and computes the final out-projection locally with the
    full Wo (no partial sums, no fp32 reduce). Wire bytes drop ~8x vs the
    fp32 ReduceScatter whose ~230us stream was the old critical path.
  - Score matmuls pack the two heads of an mc-chunk as two concurrent K=64
    row-group tiles (partitions 0-63 / 64-127 -> tile_position (0,0)/(64,0)),
    writing one [128, 2*512] 2-bank PSUM pair tile.
  - One exp activation per pair tile ([128,1024], (1024+352)/1.2 = 1.15us)
    instead of two [128,512] ones; the kt loop is software-pipelined
    (scores kt+1 emitted before AV kt) so the in-order PE queue never blocks
    the ACT exp stream. ACT does nothing but exp (qT bias + yT evacuation
    moved to DVE); the Exp table is pre-loaded by a warmup activation at
    kernel start.
  - Mask multiply applies to both heads in one [128,1024] DVE op (mask tile
    duplicated host-side).

Dtypes: all matmul operands bf16 with fp32 PSUM accumulation; softmax exp(s)
without row-max (scores O(1), scale folded into Wq host-side); normalization
via per-span stacked reciprocal + PE rank-1 broadcast, applied in-place in
yT_s one span behind attention (keeps the DVE chain off the PE critical path).
"""

import os
import numpy as np
import ml_dtypes

BF16 = ml_dtypes.bfloat16

B, T, D, H = 2, 2048, 1024, 16
HD = D // H                     # 64
NCORES = 8
GROUPS = 4                      # cores per batch (tensor-parallel degree)
HL = H // GROUPS                # heads per core = 4
DL = D // GROUPS                # dims per core = 256
SP = 512                        # free-dim span per matmul (one PSUM bank, fp32)
QS = T // SP                    # 4 q spans
KT = T // 128                   # 16 k tiles
QB = 64                         # q columns per rank territory per span
SCALE = HD ** -0.5

_CACHE = {}


def _build_program():
    import concourse.bass as bass  # noqa: F401  (registers bass machinery)
    import concourse.tile as tile
    from concourse import bacc, mybir

    f32 = mybir.dt.float32
    bf16 = mybir.dt.bfloat16
    Exp = mybir.ActivationFunctionType.Exp

    nc = bacc.Bacc("TRN2", target_bir_lowering=False, debug=False,
                   num_devices=NCORES)

    xT = nc.dram_tensor("xT", [D, T], bf16, kind="ExternalInput")
    wqT = nc.dram_tensor("wqT", [D, DL], bf16, kind="ExternalInput")
    wkT = nc.dram_tensor("wkT", [D, DL], bf16, kind="ExternalInput")
    wvT = nc.dram_tensor("wvT", [D, DL], bf16, kind="ExternalInput")
    woT = nc.dram_tensor("woT", [D, D], bf16, kind="ExternalInput")
    bqP = nc.dram_tensor("bqP", [128, 2], f32, kind="ExternalInput")
    bkP = nc.dram_tensor("bkP", [128, 2], f32, kind="ExternalInput")
    bv = nc.dram_tensor("bv", [1, DL], bf16, kind="ExternalInput")
    bo = nc.dram_tensor("bo", [1, D], bf16, kind="ExternalInput")
    maskd = nc.dram_tensor("maskd", [KT, 128, 2 * SP], bf16,
                           kind="ExternalInput")
    onesb = nc.dram_tensor("onesb", [1, 64], bf16, kind="ExternalInput")
    out_ext = nc.dram_tensor("out", [QS, 128, D], f32, kind="ExternalOutput")

    ALL8 = [[0, 1, 2, 3, 4, 5, 6, 7]]

    with tile.TileContext(nc) as tc:
        with tc.tile_pool(name="main", bufs=1) as main, \
             tc.tile_pool(name="dram", bufs=1, space="DRAM") as dram:
            qT_s = main.tile([128, 2, T], bf16)
            kT_s = main.tile([128, 2, T], bf16)
            v_s = main.tile([128, KT, HL * 65], bf16)
            yT_s = main.tile([128, 2, T], bf16)
            woT_s = main.tile([128, 8, D], bf16)
            bq_s = main.tile([128, 2], f32)
            bk_s = main.tile([128, 2], f32)
            bv_s = main.tile([1, DL], bf16)
            bo_s = main.tile([1, D], bf16)
            onesb_s = main.tile([1, 64], bf16)
            bo_bc = main.tile([128, D], bf16)
            bv_bc = main.tile([128, DL], bf16)
            maskd_s = main.tile([128, KT, 2 * SP], bf16)
            warm_s = main.tile([128, 2], f32)

            # per-span A2A staging (separate tiles avoid false DRAM deps)
            a2a_in = [dram.tile([8, DL, QB], bf16, name=f"a2ain{i}")
                      for i in range(QS)]
            a2a_out = [dram.tile([8, DL, QB], bf16, name=f"a2aout{i}")
                       for i in range(QS)]

            # tiny high-priority loads on the sync queue
            nc.sync.dma_start(out=bq_s, in_=bqP[:])
            nc.sync.dma_start(out=bk_s, in_=bkP[:])
            # pre-load the ACT Exp table during phase-1 DMAs so span 0's
            # first real exp doesn't pay the ~2.7us table switch
            nc.scalar.activation(warm_s, bq_s, Exp)
            # small loads on the scalar queue
            nc.scalar.dma_start(out=onesb_s, in_=onesb[:])
            nc.scalar.dma_start(out=bv_bc, in_=bv[:].to_broadcast([128, DL]))
            nc.scalar.dma_start(out=bo_bc, in_=bo[:].to_broadcast([128, D]))
            # ones column at index 64 of each head's 65-wide block of v_aug
            nc.vector.memset(v_s, 1.0)

            # ---------------- phase 1: projections ----------------
            with tc.tile_pool(name="proj", bufs=1) as proj, \
                 tc.tile_pool(name="pj_psum", bufs=3, space="PSUM") as pj_psum:
                xt_s = proj.tile([128, 8, T], bf16)
                wq_s = proj.tile([128, 8, DL], bf16)
                wk_s = proj.tile([128, 8, DL], bf16)
                wv_s = proj.tile([128, 8, DL], bf16)

                # critical path first: wq then the x chunks (split across the
                # sync and gpsimd queues); wk/wv follow behind x on gpsimd
                wq_r = wqT[:].rearrange("(c p) n -> c p n", p=128)
                for c in range(8):
                    nc.sync.dma_start(out=wq_s[:, c, :], in_=wq_r[c])
                xT_r = xT[:].rearrange("(c p) t -> c p t", p=128)
                for c in range(8):
                    eng = nc.sync if c % 2 == 0 else nc.gpsimd
                    eng.dma_start(out=xt_s[:, c, :], in_=xT_r[c])
                # wk/wv on the scalar queue (needed only after qT finishes),
                # followed by the attention/outproj bulk loads
                for w_s, w_d in ((wk_s, wkT), (wv_s, wvT)):
                    w_r = w_d[:].rearrange("(c p) n -> c p n", p=128)
                    for c in range(8):
                        nc.scalar.dma_start(out=w_s[:, c, :], in_=w_r[c])
                for i in range(KT):
                    nc.scalar.dma_start(out=maskd_s[:, i, :], in_=maskd[i])
                woT_r = woT[:].rearrange("(c p) n -> c p n", p=128)
                for c in range(8):
                    nc.scalar.dma_start(out=woT_s[:, c, :], in_=woT_r[c])

                # qT / kT: out[dims-chunk, t-span]; bias added on DVE during
                # the PSUM->SBUF evacuation (ACT stays exp-only)
                for w_s, b_s, dst in ((wq_s, bq_s, qT_s),
                                      (wk_s, bk_s, kT_s)):
                    for mc in range(2):
                        for s in range(QS):
                            ps = pj_psum.tile([128, SP], f32, tag="pj")
                            for kc in range(8):
                                nc.tensor.matmul(
                                    ps,
                                    lhsT=w_s[:, kc, mc * 128:(mc + 1) * 128],
                                    rhs=xt_s[:, kc, s * SP:(s + 1) * SP],
                                    start=(kc == 0), stop=(kc == 7))
                            nc.vector.tensor_scalar_add(
                                dst[:, mc, s * SP:(s + 1) * SP],
                                ps, b_s[:, mc:mc + 1])

                # v: natural layout; bias via broadcast add on DVE
                for mt in range(KT):
                    ps = pj_psum.tile([128, DL], f32, tag="pjv")
                    for kc in range(8):
                        nc.tensor.matmul(
                            ps,
                            lhsT=xt_s[:, kc, mt * 128:(mt + 1) * 128],
                            rhs=wv_s[:, kc, :],
                            start=(kc == 0), stop=(kc == 7))
                    nc.vector.tensor_add(
                        v_s[:, mt, :].rearrange(
                            "p (h d) -> p h d", d=65)[:, :, 0:64],
                        ps.rearrange("p (h d) -> p h d", d=64),
                        bv_bc.rearrange("p (h d) -> p h d", d=64))

            # ---- phase 2/3: per-span attention, pipelined with normalize +
            # A2A one span behind and out-projection two spans behind
            with tc.tile_pool(name="attn_t", bufs=4) as attn_t, \
                 tc.tile_pool(name="nrm", bufs=2) as nrm, \
                 tc.tile_pool(name="op_sb", bufs=4) as op_sb, \
                 tc.tile_pool(name="sc_psum", bufs=2, space="PSUM") as sc_psum, \
                 tc.tile_pool(name="av_psum", bufs=2, space="PSUM") as av_psum, \
                 tc.tile_pool(name="op_psum", bufs=2, space="PSUM") as op_psum:

                def attention_span(qs):
                    # denominator rows at partitions 0/32/64/96 (engine APs
                    # must start 32-aligned); memset keeps unused rows finite
                    den_stack = nrm.tile([97, SP], f32, tag="den")
                    nc.vector.memset(den_stack, 1.0)
                    nkt = 4 * qs + 4  # causal: later k tiles are all-masked
                    span = slice(qs * SP, (qs + 1) * SP)

                    for p in range(2):  # head pair = mc chunk p
                        qa = qT_s[0:64, p, span]
                        qb = qT_s[64:128, p, span]
                        ya = av_psum.tile([65, SP], f32, tag="av")
                        yb = av_psum.tile([65, SP], f32, tag="av")

                        def sc_pair(kt):
                            scp = sc_psum.tile([128, 2 * SP], f32, tag="sc")
                            nc.tensor.matmul(
                                scp[:, 0:SP],
                                lhsT=kT_s[0:64, p, kt * 128:(kt + 1) * 128],
                                rhs=qa, start=True, stop=True)
                            nc.tensor.matmul(
                                scp[:, SP:2 * SP],
                                lhsT=kT_s[64:128, p, kt * 128:(kt + 1) * 128],
                                rhs=qb, start=True, stop=True)
                            return scp

                        # software pipeline: scores kt+1 are emitted before
                        # the AV matmuls of kt so the in-order PE queue keeps
                        # feeding ACT while AV waits on exp kt
                        scp = sc_pair(0)
                        for kt in range(nkt):
                            atp = attn_t.tile([128, 2 * SP], bf16, tag="at")
                            nc.scalar.activation(atp, scp, Exp)
                            if kt >= 4 * qs:  # diagonal tile: apply mask
                                nc.vector.tensor_mul(atp, atp,
                                                     maskd_s[:, kt, :])
                            if kt + 1 < nkt:
                                scp = sc_pair(kt + 1)
                            nc.tensor.matmul(
                                ya, lhsT=v_s[:, kt, (2 * p) * 65:
                                             (2 * p + 1) * 65],
                                rhs=atp[:, 0:SP],
                                start=(kt == 0), stop=(kt == nkt - 1))
                            nc.tensor.matmul(
                                yb, lhsT=v_s[:, kt, (2 * p + 1) * 65:
                                             (2 * p + 2) * 65],
                                rhs=atp[:, SP:2 * SP],
                                start=(kt == 0), stop=(kt == nkt - 1))
                        # evacuate unnormalized yT' + denominators on DVE so
                        # the PSUM banks free for the next pair
                        nc.vector.tensor_copy(yT_s[0:64, p, span], ya[0:64, :])
                        nc.vector.tensor_copy(yT_s[64:128, p, span],
                                              yb[0:64, :])
                        nc.vector.tensor_copy(
                            den_stack[64 * p:64 * p + 1, :], ya[64:65, :])
                        nc.vector.tensor_copy(
                            den_stack[64 * p + 32:64 * p + 33, :],
                            yb[64:65, :])

                    # pure-DVE tail: reciprocal + per-head bf16 rows for the
                    # PE broadcast (consumed one span later)
                    rec_f = nrm.tile([97, SP], f32, tag="recf")
                    nc.vector.reciprocal(rec_f, den_stack)
                    rec_hs = []
                    for h in range(HL):
                        rec_h = nrm.tile([1, SP], bf16, tag="rech", bufs=8)
                        nc.vector.tensor_copy(rec_h,
                                              rec_f[32 * h:32 * h + 1, :])
                        rec_hs.append(rec_h)
                    return rec_hs

                def norm_a2a(qs, rec_hs):
                    # broadcast 1/denom across partitions on the PE, then
                    # normalize yT in place
                    span = slice(qs * SP, (qs + 1) * SP)
                    for h in range(HL):
                        mc, r0 = divmod(h, 2)
                        r0 *= 64
                        rb = op_psum.tile([64, SP], f32, tag="op")
                        nc.tensor.matmul(rb, lhsT=onesb_s[0:1, 0:64],
                                         rhs=rec_hs[h], start=True, stop=True)
                        yv = yT_s[r0:r0 + 64, mc, span]
                        nc.vector.tensor_mul(yv, yv, rb)
                    # stage my dims-slice for each rank's 64-col territory,
                    # then exchange: shard j rows = (mc*128 + p) local dims
                    in_r = a2a_in[qs][:].rearrange("j (two p) q -> two p j q",
                                                   p=128)
                    for mc in range(2):
                        nc.sync.dma_start(
                            out=in_r[mc],
                            in_=yT_s[:, mc, span].rearrange(
                                "p (j q) -> p j q", q=QB))
                    nc.gpsimd.collective_compute(
                        "AllToAll", mybir.AluOpType.bypass,
                        replica_groups=ALL8,
                        ins=[a2a_in[qs][:].opt()],
                        outs=[a2a_out[qs][:].opt()])

                def outproj(qs):
                    # shard from rank j = b*4+j2 half h holds global dims
                    # chunk kc = j2*2+h of batch b, for my 64 q columns
                    yg = op_sb.tile([128, 8, 2, QB], bf16, tag="yg")
                    out_r = a2a_out[qs][:].rearrange(
                        "(b j2) (h p) q -> b p (j2 h) q", j2=4, p=128)
                    for b in range(2):
                        nc.sync.dma_start(out=yg[:, :, b, :], in_=out_r[b])
                    for b in range(2):
                        for ns in range(2):
                            po = op_psum.tile([64, SP], f32, tag="op")
                            for kc in range(8):
                                nc.tensor.matmul(
                                    po, lhsT=yg[:, kc, b, :],
                                    rhs=woT_s[:, kc, ns * SP:(ns + 1) * SP],
                                    start=(kc == 0), stop=(kc == 7))
                            ob = op_sb.tile([64, SP], f32, tag="ob")
                            nc.vector.tensor_add(
                                ob, po, bo_bc[0:64, ns * SP:(ns + 1) * SP])
                            nc.sync.dma_start(
                                out=out_ext[qs, b * 64:(b + 1) * 64,
                                            ns * SP:(ns + 1) * SP],
                                in_=ob)

                prev_norm = None
                prev_op = None
                for qs in range(QS):
                    rec_hs = attention_span(qs)
                    if prev_norm is not None:
                        norm_a2a(*prev_norm)
                    if prev_op is not None:
                        outproj(prev_op)
                    prev_op = prev_norm[0] if prev_norm is not None else None
                    prev_norm = (qs, rec_hs)
                norm_a2a(*prev_norm)
                if prev_op is not None:
                    outproj(prev_op)
                outproj(prev_norm[0])

    nc.compile()
    return nc


def _get_program():
    if "nc" not in _CACHE:
        _CACHE["nc"] = _build_program()
    return _CACHE["nc"]


def _make_in_maps(x, mask, Wq, bq, Wk, bk, Wv, bv, Wo, bo):
    x = np.asarray(x, np.float32)
    mask = np.asarray(mask, bool)
    Wq = np.asarray(Wq, np.float32)
    Wk = np.asarray(Wk, np.float32)
    Wv = np.asarray(Wv, np.float32)
    Wo = np.asarray(Wo, np.float32)
    bq = np.asarray(bq, np.float32)
    bk = np.asarray(bk, np.float32)
    bv = np.asarray(bv, np.float32)
    bo = np.asarray(bo, np.float32)

    woT = np.ascontiguousarray(Wo.T).astype(BF16)
    in_maps = []
    per_batch = {}
    for b in range(B):
        xTb = np.ascontiguousarray(x[b].T)
        # diagonal mask tiles of mask[b,0].T, duplicated along the free dim
        # so one [128,1024] DVE op masks both heads of a pair: index
        # qs*4+j holds maskT[128*(4qs+j) : +128, 512*qs : +512] twice
        mT = mask[b, 0].T
        md = np.empty((KT, 128, 2 * SP), np.float32)
        for qs in range(QS):
            for j in range(4):
                kt = 4 * qs + j
                blk = mT[kt * 128:(kt + 1) * 128,
                         qs * SP:(qs + 1) * SP].astype(np.float32)
                md[kt, :, 0:SP] = blk
                md[kt, :, SP:2 * SP] = blk
        per_batch[b] = (xTb, md)
    for c in range(NCORES):
        b, g = divmod(c, GROUPS)
        sl = slice(g * DL, (g + 1) * DL)
        xTb, md = per_batch[b]
        in_maps.append({
            "xT": xTb.astype(BF16),
            "wqT": np.ascontiguousarray((Wq[sl] * SCALE).T).astype(BF16),
            "wkT": np.ascontiguousarray(Wk[sl].T).astype(BF16),
            "wvT": np.ascontiguousarray(Wv[sl].T).astype(BF16),
            "woT": woT,
            "bqP": np.ascontiguousarray((bq[sl] * SCALE).reshape(2, 128).T),
            "bkP": np.ascontiguousarray(bk[sl].reshape(2, 128).T),
            "bv": bv[sl].reshape(1, DL).astype(BF16),
            "bo": bo.reshape(1, D).astype(BF16),
            "maskd": md.astype(BF16),
            "onesb": np.ones((1, 64), BF16),
        })
    return in_maps


def _capture_profile(nc, in_maps, tmpdir):
    """Run with NTFF capture and process the profile ourselves (the stock
    trace path can't handle the duplicate-executable NTFFs the axon relay
    produces). Returns (results, exec_time_ns|None)."""
    import glob
    import json
    import re
    import subprocess
    from trn_agent_boot.trn_boot import _ntff_profile_via_ctypes
    from concourse import bass2jax

    hook = _ntff_profile_via_ctypes("/opt/axon/libaxon_pjrt.so")
    if hook is None:
        raise RuntimeError("libaxon_pjrt.so lacks NTFF profile symbols")
    os.makedirs(tmpdir, exist_ok=True)
    with hook(tmpdir, [0]):
        results = bass2jax.run_bass_via_pjrt(nc, in_maps, n_cores=NCORES)

    # group NTFF/NEFF pairs by executable id; use the newest executable
    ntffs = glob.glob(os.path.join(tmpdir, "*_body*-device*.ntff"))
    best, best_id = None, -1
    for f in ntffs:
        m = re.search(r"executable(\d+)-device000000", f)
        if m and int(m.group(1)) > best_id:
            best_id, best = int(m.group(1)), f
    if best is None:
        raise RuntimeError(f"no NTFF produced in {tmpdir}")
    neff = re.sub(r"-device\d+-execution-\d+\.ntff$", ".neff", best)
    out_json = os.path.join(tmpdir, "prof.json")
    subprocess.check_call(
        ["neuron-profile", "view", "--ignore-nc-buf-usage", "-s", best,
         "-n", neff, "--output-format=json", f"--output-file={out_json}"],
        cwd=tmpdir)
    summary = json.load(open(out_json))["summary"][0]
    return results, int(summary["total_time"] * 1e9)


def kernel(x, mask, Wq, bq, Wk, bk, Wv, bv, Wo, bo):
    from concourse import bass_utils

    in_maps = _make_in_maps(x, mask, Wq, bq, Wk, bk, Wv, bv, Wo, bo)
    nc = _get_program()

    trace = bool(int(os.environ.get("MHA_TRACE", "0")))
    tmpdir = os.environ.get("MHA_TRACE_DIR") or None
    results = None
    if trace and tmpdir:
        try:
            results, exec_ns = _capture_profile(nc, in_maps, tmpdir)
            _CACHE["last_exec_time_ns"] = exec_ns
        except Exception as e:  # profiling is best-effort
            print(f"profiling unavailable: {type(e).__name__}: {e}")
            results = None
    if results is None:
        results = bass_utils.run_bass_kernel_spmd(
            nc, in_maps, core_ids=list(range(NCORES))).results
        _CACHE.setdefault("last_exec_time_ns", None)

    # core c's out[qs] holds rows (q = qs*512 + c*64 + i) for batch 0
    # (rows 0-63) and batch 1 (rows 64-127)
    out = np.empty((B, T, D), np.float32)
    for c in range(NCORES):
        o = results[c]["out"]
        for qs in range(QS):
            q0 = qs * SP + c * QB
            out[0, q0:q0 + QB] = o[qs, 0:QB]
            out[1, q0:q0 + QB] = o[qs, QB:2 * QB]
    return out


# revision 16
# speedup vs baseline: 1.2817x; 1.0173x over previous
"""Causal multi-head attention (B=2, T=2048, D=1024, H=16) on 8 TRN2 NeuronCores.

Sharding: core c = (batch b = c//4, head-group g = c%4). Each core owns 4 heads
(= 256 contiguous dims of D) of one batch: Megatron-style tensor parallelism on
heads x data parallelism on batch.

v2 changes over the ReduceScatter baseline (352us):
  - Out-projection reduction replaced by a per-q-span 8-way bf16 AllToAll of
    the normalized attention output yT. Rank r's "territory" is the 64-col
    q-block r of each span, for BOTH batches; every core has real data for
    every peer (its dims-slice of its batch), so the exchange is SPMD-uniform
    with no junk shards. After the A2A each core holds full-D yT for its 64 q
    columns x 2 batches and computes the final out-projection locally with the
    full Wo (no partial sums, no fp32 reduce). Wire bytes drop ~8x vs the
    fp32 ReduceScatter whose ~230us stream was the old critical path.
  - Score matmuls pack the two heads of an mc-chunk as two concurrent K=64
    row-group tiles (partitions 0-63 / 64-127 -> tile_position (0,0)/(64,0)),
    writing one [128, 2*512] 2-bank PSUM pair tile.
  - One exp activation per pair tile ([128,1024], (1024+352)/1.2 = 1.15us)
    instead of two [128,512] ones; the kt loop is software-pipelined
    (scores kt+1 emitted before AV kt) so the in-order PE queue never blocks
    the ACT exp stream. ACT does nothing but exp (qT bias + yT evacuation
    moved to DVE); the Exp table is pre-loaded by a warmup activation at
    kernel start.
  - Mask multiply applies to both heads in one [128,1024] DVE op (mask tile
    duplicated host-side).

Dtypes: all matmul operands bf16 with fp32 PSUM accumulation; softmax exp(s)
without row-max (scores O(1), scale folded into Wq host-side); normalization
via per-span stacked reciprocal + PE rank-1 broadcast, applied in-place in
yT_s one span behind attention (keeps the DVE chain off the PE critical path).
"""

import os
import numpy as np
import ml_dtypes

BF16 = ml_dtypes.bfloat16

B, T, D, H = 2, 2048, 1024, 16
HD = D // H                     # 64
NCORES = 8
GROUPS = 4                      # cores per batch (tensor-parallel degree)
HL = H // GROUPS                # heads per core = 4
DL = D // GROUPS                # dims per core = 256
SP = 512                        # free-dim span per matmul (one PSUM bank, fp32)
QS = T // SP                    # 4 q spans
KT = T // 128                   # 16 k tiles
QB = 64                         # q columns per rank territory per span
SCALE = HD ** -0.5

_CACHE = {}


def _build_program():
    import concourse.bass as bass  # noqa: F401  (registers bass machinery)
    import concourse.tile as tile
    from concourse import bacc, mybir

    f32 = mybir.dt.float32
    bf16 = mybir.dt.bfloat16
    Exp = mybir.ActivationFunctionType.Exp
    Ln = mybir.ActivationFunctionType.Ln

    nc = bacc.Bacc("TRN2", target_bir_lowering=False, debug=False,
                   num_devices=NCORES)

    xT = nc.dram_tensor("xT", [D, T], bf16, kind="ExternalInput")
    wqT = nc.dram_tensor("wqT", [D, DL], bf16, kind="ExternalInput")
    wkT = nc.dram_tensor("wkT", [D, DL], bf16, kind="ExternalInput")
    wvT = nc.dram_tensor("wvT", [D, DL], bf16, kind="ExternalInput")
    woT = nc.dram_tensor("woT", [D, D], bf16, kind="ExternalInput")
    bqP = nc.dram_tensor("bqP", [128, 2], f32, kind="ExternalInput")
    bkP = nc.dram_tensor("bkP", [128, 2], f32, kind="ExternalInput")
    bv = nc.dram_tensor("bv", [1, DL], bf16, kind="ExternalInput")
    bo = nc.dram_tensor("bo", [1, D], bf16, kind="ExternalInput")
    maskd = nc.dram_tensor("maskd", [KT, 128, 2 * SP], bf16,
                           kind="ExternalInput")
    onesb = nc.dram_tensor("onesb", [1, 64], bf16, kind="ExternalInput")
    out_ext = nc.dram_tensor("out", [QS, 128, D], f32, kind="ExternalOutput")

    ALL8 = [[0, 1, 2, 3, 4, 5, 6, 7]]

    with tile.TileContext(nc) as tc:
        with tc.tile_pool(name="main", bufs=1) as main, \
             tc.tile_pool(name="dram", bufs=1, space="DRAM") as dram:
            qT_s = main.tile([128, 2, T], bf16)
            kT_s = main.tile([128, 2, T], bf16)
            v_s = main.tile([128, KT, HL * 65], bf16)
            yT_s = main.tile([128, 2, T], bf16)
            woT_s = main.tile([128, 8, D], bf16)
            bq_s = main.tile([128, 2], f32)
            bk_s = main.tile([128, 2], f32)
            bv_s = main.tile([1, DL], bf16)
            bo_s = main.tile([1, D], bf16)
            onesb_s = main.tile([128, 64], bf16)
            bv_bc = main.tile([128, DL], bf16)
            maskd_s = main.tile([128, KT, 2 * SP], bf16)
            warm_s = main.tile([128, 2], f32)
            warm_sb = main.tile([128, SP], bf16)

            # per-span A2A staging (separate tiles avoid false DRAM deps)
            a2a_in = [dram.tile([8, DL, QB], bf16, name=f"a2ain{i}")
                      for i in range(QS)]
            a2a_out = [dram.tile([8, DL, QB], bf16, name=f"a2aout{i}")
                       for i in range(QS)]
            dummy_in = dram.tile([8, 16], bf16, name="dummyin")
            dummy_out = dram.tile([8, 16], bf16, name="dummyout")

            # PE warmup: ~15us of back-to-back matmuls on scratch data while
            # the input DMAs stream in, so HAM un-throttles (K=4/8 -> 8/8)
            # before the first real projection matmul
            nc.vector.memset(warm_sb, 1.0)
            with tc.tile_pool(name="warm_psum", bufs=1,
                              space="PSUM") as warm_psum:
                wps = warm_psum.tile([128, SP], f32, tag="w")
                for _ in range(36):
                    nc.tensor.matmul(wps, lhsT=warm_sb[:, 0:128],
                                     rhs=warm_sb, start=True, stop=True)

            # tiny high-priority loads on the sync queue
            nc.sync.dma_start(out=bq_s, in_=bqP[:])
            nc.sync.dma_start(out=bk_s, in_=bkP[:])
            # pre-load the ACT Log+Exp table during phase-1 DMAs so span 0's
            # first real exp doesn't pay the ~2.7us table switch
            nc.scalar.activation(warm_s, warm_sb[:, 0:2], Ln)
            nc.scalar.activation(warm_s, warm_sb[:, 0:2], Exp)
            # small loads on the scalar queue
            nc.scalar.dma_start(out=onesb_s,
                                in_=onesb[:].to_broadcast([128, 64]))
            nc.scalar.dma_start(out=bv_bc, in_=bv[:].to_broadcast([128, DL]))
            nc.scalar.dma_start(out=bo_s, in_=bo[:])
            # ones column at index 64 of each head's 65-wide block of v_aug
            nc.vector.memset(v_s, 1.0)

            # ---------------- phase 1: projections ----------------
            with tc.tile_pool(name="proj", bufs=1) as proj, \
                 tc.tile_pool(name="pj_psum", bufs=3, space="PSUM") as pj_psum:
                xt_s = proj.tile([128, 8, T], bf16)
                wq_s = proj.tile([128, 8, DL], bf16)
                wk_s = proj.tile([128, 8, DL], bf16)
                wv_s = proj.tile([128, 8, DL], bf16)

                # critical path first, in kc consumption order: the first qT
                # matmul needs wq[0] + x[0]; interleave so chunk kc lands
                # roughly in order (x odd chunks on the gpsimd queue)
                wq_r = wqT[:].rearrange("(c p) n -> c p n", p=128)
                xT_r = xT[:].rearrange("(c p) t -> c p t", p=128)
                for c in range(8):
                    nc.sync.dma_start(out=wq_s[:, c, :], in_=wq_r[c])
                    if c % 2 == 0:
                        nc.sync.dma_start(out=xt_s[:, c, :], in_=xT_r[c])
                    else:
                        nc.gpsimd.dma_start(out=xt_s[:, c, :], in_=xT_r[c])
                # warm up the collectives mesh path so the first real
                # AllToAll doesn't pay the first-collective wind-up
                nc.gpsimd.collective_compute(
                    "AllToAll", mybir.AluOpType.bypass, replica_groups=ALL8,
                    ins=[dummy_in[:].opt()], outs=[dummy_out[:].opt()])
                # wk/wv on the scalar queue (needed only after qT finishes),
                # followed by the attention/outproj bulk loads
                for w_s, w_d in ((wk_s, wkT), (wv_s, wvT)):
                    w_r = w_d[:].rearrange("(c p) n -> c p n", p=128)
                    for c in range(8):
                        nc.scalar.dma_start(out=w_s[:, c, :], in_=w_r[c])
                for i in range(KT):
                    nc.scalar.dma_start(out=maskd_s[:, i, :], in_=maskd[i])
                woT_r = woT[:].rearrange("(c p) n -> c p n", p=128)
                for c in range(8):
                    nc.scalar.dma_start(out=woT_s[:, c, :], in_=woT_r[c])

                # qT / kT: out[dims-chunk, t-span]; bias added on DVE during
                # the PSUM->SBUF evacuation (ACT stays exp-only)
                for w_s, b_s, dst in ((wq_s, bq_s, qT_s),
                                      (wk_s, bk_s, kT_s)):
                    for mc in range(2):
                        for s in range(QS):
                            ps = pj_psum.tile([128, SP], f32, tag="pj")
                            for kc in range(8):
                                nc.tensor.matmul(
                                    ps,
                                    lhsT=w_s[:, kc, mc * 128:(mc + 1) * 128],
                                    rhs=xt_s[:, kc, s * SP:(s + 1) * SP],
                                    start=(kc == 0), stop=(kc == 7))
                            nc.vector.tensor_scalar_add(
                                dst[:, mc, s * SP:(s + 1) * SP],
                                ps, b_s[:, mc:mc + 1])

                # v: natural layout; bias via broadcast add on DVE
                for mt in range(KT):
                    ps = pj_psum.tile([128, DL], f32, tag="pjv")
                    for kc in range(8):
                        nc.tensor.matmul(
                            ps,
                            lhsT=xt_s[:, kc, mt * 128:(mt + 1) * 128],
                            rhs=wv_s[:, kc, :],
                            start=(kc == 0), stop=(kc == 7))
                    nc.vector.tensor_add(
                        v_s[:, mt, :].rearrange(
                            "p (h d) -> p h d", d=65)[:, :, 0:64],
                        ps.rearrange("p (h d) -> p h d", d=64),
                        bv_bc.rearrange("p (h d) -> p h d", d=64))

            # ---- phase 2/3: per-span attention, pipelined with normalize +
            # A2A one span behind and out-projection two spans behind
            with tc.tile_pool(name="attn_t", bufs=4) as attn_t, \
                 tc.tile_pool(name="nrm", bufs=2) as nrm, \
                 tc.tile_pool(name="op_sb", bufs=4) as op_sb, \
                 tc.tile_pool(name="sc_psum", bufs=2, space="PSUM") as sc_psum, \
                 tc.tile_pool(name="av_psum", bufs=2, space="PSUM") as av_psum, \
                 tc.tile_pool(name="op_psum", bufs=2, space="PSUM") as op_psum:

                def attention_span(qs):
                    # denominator rows at partitions 0/32/64/96 (engine APs
                    # must start 32-aligned); memset keeps unused rows finite
                    den_stack = nrm.tile([97, SP], f32, tag="den")
                    nc.vector.memset(den_stack, 1.0)
                    nkt = 4 * qs + 4  # causal: later k tiles are all-masked
                    span = slice(qs * SP, (qs + 1) * SP)

                    for p in range(2):  # head pair = mc chunk p
                        qa = qT_s[0:64, p, span]
                        qb = qT_s[64:128, p, span]
                        ya = av_psum.tile([65, SP], f32, tag="av")
                        yb = av_psum.tile([65, SP], f32, tag="av")

                        def sc_pair(kt):
                            scp = sc_psum.tile([128, 2 * SP], f32, tag="sc")
                            nc.tensor.matmul(
                                scp[:, 0:SP],
                                lhsT=kT_s[0:64, p, kt * 128:(kt + 1) * 128],
                                rhs=qa, start=True, stop=True)
                            nc.tensor.matmul(
                                scp[:, SP:2 * SP],
                                lhsT=kT_s[64:128, p, kt * 128:(kt + 1) * 128],
                                rhs=qb, start=True, stop=True)
                            return scp

                        # software pipeline: scores kt+1 are emitted before
                        # the AV matmuls of kt so the in-order PE queue keeps
                        # feeding ACT while AV waits on exp kt
                        scp = sc_pair(0)
                        for kt in range(nkt):
                            atp = attn_t.tile([128, 2 * SP], bf16, tag="at")
                            nc.scalar.activation(atp, scp, Exp)
                            if kt >= 4 * qs:  # diagonal tile: apply mask
                                nc.vector.tensor_mul(atp, atp,
                                                     maskd_s[:, kt, :])
                            if kt + 1 < nkt:
                                scp = sc_pair(kt + 1)
                            nc.tensor.matmul(
                                ya, lhsT=v_s[:, kt, (2 * p) * 65:
                                             (2 * p + 1) * 65],
                                rhs=atp[:, 0:SP],
                                start=(kt == 0), stop=(kt == nkt - 1))
                            nc.tensor.matmul(
                                yb, lhsT=v_s[:, kt, (2 * p + 1) * 65:
                                             (2 * p + 2) * 65],
                                rhs=atp[:, SP:2 * SP],
                                start=(kt == 0), stop=(kt == nkt - 1))
                        # evacuate unnormalized yT' + denominators on DVE so
                        # the PSUM banks free for the next pair
                        nc.vector.tensor_copy(yT_s[0:64, p, span], ya[0:64, :])
                        nc.vector.tensor_copy(yT_s[64:128, p, span],
                                              yb[0:64, :])
                        nc.vector.tensor_copy(
                            den_stack[64 * p:64 * p + 1, :], ya[64:65, :])
                        nc.vector.tensor_copy(
                            den_stack[64 * p + 32:64 * p + 33, :],
                            yb[64:65, :])

                    # 1/den as exp(-log(den)) on ACT (Log+Exp share one table
                    # set; DVE's iterative reciprocal costs 3.4us per span).
                    # Runs in ACT's span-boundary gap; consumed a span later.
                    lg = nrm.tile([97, SP], f32, tag="lg")
                    nc.scalar.activation(lg, den_stack, Ln)
                    rec_bf = nrm.tile([97, SP], bf16, tag="recf")
                    nc.scalar.activation(rec_bf, lg, Exp, scale=-1.0)
                    return [rec_bf[32 * h:32 * h + 1, :] for h in range(HL)]

                def norm_a2a(qs, rec_hs):
                    # broadcast 1/denom across partitions on the PE, then
                    # normalize yT in place
                    span = slice(qs * SP, (qs + 1) * SP)
                    for h in range(HL):
                        mc, r0 = divmod(h, 2)
                        r0 *= 64
                        rb = op_psum.tile([64, SP], f32, tag="op")
                        r0p = 32 * h  # lhsT/rhs base partitions must match
                        nc.tensor.matmul(rb, lhsT=onesb_s[r0p:r0p + 1, :],
                                         rhs=rec_hs[h], start=True, stop=True,
                                         tile_position=(r0p, 0))
                        yv = yT_s[r0:r0 + 64, mc, span]
                        nc.vector.tensor_mul(yv, yv, rb)
                    # stage my dims-slice for each rank's 64-col territory,
                    # then exchange: shard j rows = (mc*128 + p) local dims
                    in_r = a2a_in[qs][:].rearrange("j (two p) q -> two p j q",
                                                   p=128)
                    for mc in range(2):
                        nc.sync.dma_start(
                            out=in_r[mc],
                            in_=yT_s[:, mc, span].rearrange(
                                "p (j q) -> p j q", q=QB))
                    nc.gpsimd.collective_compute(
                        "AllToAll", mybir.AluOpType.bypass,
                        replica_groups=ALL8,
                        ins=[a2a_in[qs][:].opt()],
                        outs=[a2a_out[qs][:].opt()])

                def outproj(qs):
                    # shard from rank j = b*4+j2 half h holds global dims
                    # chunk kc = j2*2+h of batch b, for my 64 q columns
                    yg = op_sb.tile([128, 8, 2, QB], bf16, tag="yg")
                    out_r = a2a_out[qs][:].rearrange(
                        "(b j2) (h p) q -> b p (j2 h) q", j2=4, p=128)
                    for b in range(2):
                        nc.sync.dma_start(out=yg[:, :, b, :], in_=out_r[b])
                    for b in range(2):
                        for ns in range(2):
                            po = op_psum.tile([64, SP], f32, tag="op")
                            # rank-1 seed adds bo along the free dim; it is
                            # the only start=True matmul into this bank
                            nc.tensor.matmul(
                                po, lhsT=onesb_s[0:1, :],
                                rhs=bo_s[0:1, ns * SP:(ns + 1) * SP],
                                start=True, stop=False)
                            for kc in range(8):
                                nc.tensor.matmul(
                                    po, lhsT=yg[:, kc, b, :],
                                    rhs=woT_s[:, kc, ns * SP:(ns + 1) * SP],
                                    start=False, stop=(kc == 7))
                            # evacuate on ACT (idle at span boundaries); the
                            # copy opcode shares the Log/Exp table set
                            ob = op_sb.tile([64, SP], f32, tag="ob")
                            nc.scalar.copy(ob, po)
                            nc.sync.dma_start(
                                out=out_ext[qs, b * 64:(b + 1) * 64,
                                            ns * SP:(ns + 1) * SP],
                                in_=ob)

                prev_norm = None
                prev_op = None
                for qs in range(QS):
                    rec_hs = attention_span(qs)
                    if prev_norm is not None:
                        norm_a2a(*prev_norm)
                    if prev_op is not None:
                        outproj(prev_op)
                    prev_op = prev_norm[0] if prev_norm is not None else None
                    prev_norm = (qs, rec_hs)
                norm_a2a(*prev_norm)
                if prev_op is not None:
                    outproj(prev_op)
                outproj(prev_norm[0])

    nc.compile()
    return nc


def _get_program():
    if "nc" not in _CACHE:
        _CACHE["nc"] = _build_program()
    return _CACHE["nc"]


def _make_in_maps(x, mask, Wq, bq, Wk, bk, Wv, bv, Wo, bo):
    x = np.asarray(x, np.float32)
    mask = np.asarray(mask, bool)
    Wq = np.asarray(Wq, np.float32)
    Wk = np.asarray(Wk, np.float32)
    Wv = np.asarray(Wv, np.float32)
    Wo = np.asarray(Wo, np.float32)
    bq = np.asarray(bq, np.float32)
    bk = np.asarray(bk, np.float32)
    bv = np.asarray(bv, np.float32)
    bo = np.asarray(bo, np.float32)

    woT = np.ascontiguousarray(Wo.T).astype(BF16)
    in_maps = []
    per_batch = {}
    for b in range(B):
        xTb = np.ascontiguousarray(x[b].T)
        # diagonal mask tiles of mask[b,0].T, duplicated along the free dim
        # so one [128,1024] DVE op masks both heads of a pair: index
        # qs*4+j holds maskT[128*(4qs+j) : +128, 512*qs : +512] twice
        mT = mask[b, 0].T
        md = np.empty((KT, 128, 2 * SP), np.float32)
        for qs in range(QS):
            for j in range(4):
                kt = 4 * qs + j
                blk = mT[kt * 128:(kt + 1) * 128,
                         qs * SP:(qs + 1) * SP].astype(np.float32)
                md[kt, :, 0:SP] = blk
                md[kt, :, SP:2 * SP] = blk
        per_batch[b] = (xTb, md)
    for c in range(NCORES):
        b, g = divmod(c, GROUPS)
        sl = slice(g * DL, (g + 1) * DL)
        xTb, md = per_batch[b]
        in_maps.append({
            "xT": xTb.astype(BF16),
            "wqT": np.ascontiguousarray((Wq[sl] * SCALE).T).astype(BF16),
            "wkT": np.ascontiguousarray(Wk[sl].T).astype(BF16),
            "wvT": np.ascontiguousarray(Wv[sl].T).astype(BF16),
            "woT": woT,
            "bqP": np.ascontiguousarray((bq[sl] * SCALE).reshape(2, 128).T),
            "bkP": np.ascontiguousarray(bk[sl].reshape(2, 128).T),
            "bv": bv[sl].reshape(1, DL).astype(BF16),
            "bo": bo.reshape(1, D).astype(BF16),
            "maskd": md.astype(BF16),
            "onesb": np.ones((1, 64), BF16),
        })
    return in_maps


def _capture_profile(nc, in_maps, tmpdir):
    """Run with NTFF capture and process the profile ourselves (the stock
    trace path can't handle the duplicate-executable NTFFs the axon relay
    produces). Returns (results, exec_time_ns|None)."""
    import glob
    import json
    import re
    import subprocess
    from trn_agent_boot.trn_boot import _ntff_profile_via_ctypes
    from concourse import bass2jax

    hook = _ntff_profile_via_ctypes("/opt/axon/libaxon_pjrt.so")
    if hook is None:
        raise RuntimeError("libaxon_pjrt.so lacks NTFF profile symbols")
    os.makedirs(tmpdir, exist_ok=True)
    with hook(tmpdir, [0]):
        results = bass2jax.run_bass_via_pjrt(nc, in_maps, n_cores=NCORES)

    # group NTFF/NEFF pairs by executable id; use the newest executable
    ntffs = glob.glob(os.path.join(tmpdir, "*_body*-device*.ntff"))
    best, best_id = None, -1
    for f in ntffs:
        m = re.search(r"executable(\d+)-device000000", f)
        if m and int(m.group(1)) > best_id:
            best_id, best = int(m.group(1)), f
    if best is None:
        raise RuntimeError(f"no NTFF produced in {tmpdir}")
    neff = re.sub(r"-device\d+-execution-\d+\.ntff$", ".neff", best)
    out_json = os.path.join(tmpdir, "prof.json")
    subprocess.check_call(
        ["neuron-profile", "view", "--ignore-nc-buf-usage", "-s", best,
         "-n", neff, "--output-format=json", f"--output-file={out_json}"],
        cwd=tmpdir)
    summary = json.load(open(out_json))["summary"][0]
    return results, int(summary["total_time"] * 1e9)


def kernel(x, mask, Wq, bq, Wk, bk, Wv, bv, Wo, bo):
    from concourse import bass_utils

    in_maps = _make_in_maps(x, mask, Wq, bq, Wk, bk, Wv, bv, Wo, bo)
    nc = _get_program()

    trace = bool(int(os.environ.get("MHA_TRACE", "0")))
    tmpdir = os.environ.get("MHA_TRACE_DIR") or None
    results = None
    if trace and tmpdir:
        try:
            results, exec_ns = _capture_profile(nc, in_maps, tmpdir)
            _CACHE["last_exec_time_ns"] = exec_ns
        except Exception as e:  # profiling is best-effort
            print(f"profiling unavailable: {type(e).__name__}: {e}")
            results = None
    if results is None:
        results = bass_utils.run_bass_kernel_spmd(
            nc, in_maps, core_ids=list(range(NCORES))).results
        _CACHE.setdefault("last_exec_time_ns", None)

    # core c's out[qs] holds rows (q = qs*512 + c*64 + i) for batch 0
    # (rows 0-63) and batch 1 (rows 64-127)
    out = np.empty((B, T, D), np.float32)
    for c in range(NCORES):
        o = results[c]["out"]
        for qs in range(QS):
            q0 = qs * SP + c * QB
            out[0, q0:q0 + QB] = o[qs, 0:QB]
            out[1, q0:q0 + QB] = o[qs, QB:2 * QB]
    return out
